# revision 1
# baseline (speedup 1.0000x reference)
"""BiDirectional LSTM (B=32, T=512, D=H=512, hard_sigmoid gates, output=fwd+bwd sum)
on 8 Trainium2 NeuronCores.

Sharding: core c in 0..7 -> direction d = c//4 (0=fwd, 1=bwd), batch shard s = c%4
(8 samples each). Backward direction is realized purely in data: the host feeds the
bwd cores time-reversed x; scan outputs stack in iteration order (matching Theano
go_backwards semantics in the reference), so fwd+bwd partial outputs add at equal
step indices.

Per-core program (SPMD, identical on all cores):
  Phase 1: xz[h', t, b] = (x @ W_cat + b_cat) transposed, via PE GEMM with W tiles
           stationary, h' on PSUM partitions -> DRAM scratch (fp32).
  Phase 2: 512 sequential steps; each step: z = xz_t + U_cat.T @ h (64 128x128 bf16
           matmul-accumulates), hard_sigmoid/tanh gates on ACT+DVE, LSTM cell update,
           h history kept in SBUF, one bulk DMA of the output at the end.
"""

import numpy as np
import ml_dtypes

B, T, D, H = 32, 512, 512, 512
NCORES = 8
BC = B // 4          # 8 samples per core
KT = D // 128        # 4 k-tiles
MT = (4 * H) // 128  # 16 m-tiles (4 gates x 4 chunks)


def build(nc, Tn=T, repeat=1):
    import concourse.mybir as mybir
    from concourse.tile import TileContext

    f32 = mybir.dt.float32
    bf16 = mybir.dt.bfloat16
    AF = mybir.ActivationFunctionType
    NT = Tn * BC  # GEMM moving free size
    TBLK = 16     # recurrence xz prefetch block (steps)
    assert Tn % TBLK == 0

    xT = nc.declare_dram_parameter("xT", [KT, 128, NT], bf16, isOutput=False)
    w = nc.declare_dram_parameter("w", [KT, 128, 4 * H], bf16, isOutput=False)
    u = nc.declare_dram_parameter("u", [KT, 128, 4 * H], bf16, isOutput=False)
    bias = nc.declare_dram_parameter("bias", [128, MT], f32, isOutput=False)
    y = nc.declare_dram_parameter("y", [128, Tn, KT, BC], f32, isOutput=True)

    xz = nc.dram_tensor("xz", [4 * H, Tn, BC], f32)
    xz_m = xz.rearrange("(m p) t b -> p m t b", p=128)

    with TileContext(nc) as tc:
        with (
            tc.tile_pool(name="const", bufs=1) as cpool,
            tc.tile_pool(name="state", bufs=1) as spool,
        ):
            # Resident inputs
            xT_sb = [cpool.tile([128, NT], bf16, name=f"xT{k}", tag=f"xT{k}") for k in range(KT)]
            w_sb = [cpool.tile([128, 4 * H], bf16, name=f"w{k}", tag=f"w{k}") for k in range(KT)]
            u_sb = [cpool.tile([128, 4 * H], bf16, name=f"u{k}", tag=f"u{k}") for k in range(KT)]
            bias_sb = cpool.tile([128, MT], f32, name="bias", tag="bias")
            for k in range(KT):
                nc.sync.dma_start(out=xT_sb[k], in_=xT[k])
                nc.sync.dma_start(out=w_sb[k], in_=w[k])
                nc.sync.dma_start(out=u_sb[k], in_=u[k])
            nc.sync.dma_start(out=bias_sb, in_=bias[:])

            y_hist = spool.tile([128, Tn, KT, BC], f32, name="y_hist", tag="y_hist")
            h_bf = spool.tile([128, KT, BC], bf16, name="h_bf", tag="h_bf")
            c_st = spool.tile([128, KT, BC], f32, name="c_st", tag="c_st")
            nc.any.memzero(h_bf)
            nc.any.memzero(c_st)
            half = cpool.tile([128, 1], f32, name="half", tag="half")
            nc.gpsimd.memset(half, 0.5)

            # ---------------- Phase 1: input GEMM ----------------
            NCK = min(512, NT)   # n-chunk width
            NCH = NT // NCK      # number of n-chunks
            with (
                tc.tile_pool(name="gpsum", bufs=2, space="PSUM") as gpsum,
                tc.tile_pool(name="gstage", bufs=4) as gstage,
            ):
                ngroups = (NCH + 3) // 4
                for m in range(MT):
                    for ng in range(ngroups):
                        nsub = min(4, NCH - ng * 4)
                        psums = [
                            gpsum.tile([128, NCK], f32, name=f"gp{n}", tag=f"gp{n}")
                            for n in range(nsub)
                        ]
                        for k in range(KT):
                            for n in range(nsub):
                                nci = ng * 4 + n
                                nc.tensor.matmul(
                                    psums[n],
                                    lhsT=w_sb[k][:, m * 128 : (m + 1) * 128],
                                    rhs=xT_sb[k][:, nci * NCK : (nci + 1) * NCK],
                                    start=(k == 0),
                                    stop=(k == KT - 1),
                                )
                        for n in range(nsub):
                            nci = ng * 4 + n
                            stg = gstage.tile([128, NCK], f32, name="stg", tag="stg")
                            nc.scalar.activation(
                                stg, psums[n], AF.Identity,
                                bias=bias_sb[:, m : m + 1], scale=1.0,
                            )
                            nc.sync.dma_start(
                                out=xz_m[:, m, :, :].rearrange("p t b -> p (t b)")[
                                    :, nci * NCK : (nci + 1) * NCK
                                ],
                                in_=stg,
                            )

            # ---------------- Phase 2: recurrence ----------------
            with (
                tc.tile_pool(name="rpsum", bufs=2, space="PSUM") as rpsum,
                tc.tile_pool(name="xzblk", bufs=2) as xzpool,
                tc.tile_pool(name="ztmp", bufs=2) as zpool,
            ):
                for tb in range(repeat * (Tn // TBLK)):
                    tb = tb % (Tn // TBLK)
                    xzblk = xzpool.tile([128, MT, TBLK, BC], f32, name="xzblk", tag="xzblk")
                    # per-m DMAs: each is a single contiguous 512B-per-partition
                    # transfer -> lands on one DGE queue, keeping the number of
                    # distinct semaphores each consumer waits on small.
                    for m in range(MT):
                        nc.sync.dma_start(
                            out=xzblk[:, m],
                            in_=xz_m[:, m, tb * TBLK : (tb + 1) * TBLK, :],
                        )
                    for tr in range(TBLK):
                        t = tb * TBLK + tr
                        # U layout gate columns: [i | f | o | c], semantic order
                        # of matmul emission: i, f, c(tilde), o -- o last so the
                        # c-chain hides under o's matmuls and the step tail is
                        # only o's per-chunk epilogue.
                        psg = {
                            g: rpsum.tile([128, KT, BC], f32, name=f"ps{g}", tag=f"ps{g}")
                            for g in range(4)
                        }
                        for g in (0, 1, 3, 2):  # i, f, c~, o (layout index)
                            for mi in range(4):
                                m = g * 4 + mi
                                for k in range(KT):
                                    nc.tensor.matmul(
                                        psg[g][:, mi, :],
                                        lhsT=u_sb[k][:, m * 128 : (m + 1) * 128],
                                        rhs=h_bf[:, k, :],
                                        start=(k == 0),
                                        stop=(k == KT - 1),
                                    )
                        # i, f, c~ gates full-width (their psums finish early;
                        # c-chain then overlaps o's matmuls)
                        sig = {}
                        for g in (0, 1):
                            zt = zpool.tile([128, KT, BC], f32, name=f"z{g}", tag=f"z{g}")
                            nc.vector.tensor_add(
                                zt, psg[g], xzblk[:, g * 4 : (g + 1) * 4, tr, :]
                            )
                            rt = zpool.tile([128, KT, BC], f32, name=f"r{g}", tag=f"r{g}")
                            nc.scalar.activation(rt, zt, AF.Relu, bias=half[:, 0:1], scale=0.2)
                            nc.vector.tensor_scalar_min(rt, rt, 1.0)
                            sig[g] = rt
                        ztg = zpool.tile([128, KT, BC], f32, name="z3", tag="z3")
                        nc.vector.tensor_add(ztg, psg[3], xzblk[:, 12:16, tr, :])
                        gt = zpool.tile([128, KT, BC], f32, name="gt", tag="gt")
                        nc.scalar.activation(gt, ztg, AF.Tanh)
                        # c = f*c + i*g ; tanh(c) — overlaps o's matmuls
                        t1 = zpool.tile([128, KT, BC], f32, name="t1", tag="t1")
                        nc.vector.tensor_mul(t1, sig[1], c_st)
                        t2 = zpool.tile([128, KT, BC], f32, name="t2", tag="t2")
                        nc.vector.tensor_mul(t2, sig[0], gt)
                        nc.vector.tensor_add(c_st, t1, t2)
                        th = zpool.tile([128, KT, BC], f32, name="th", tag="th")
                        nc.scalar.activation(th, c_st, AF.Tanh)
                        # o gate (the only post-last-matmul tail), then h
                        zo = zpool.tile([128, KT, BC], f32, name="zo", tag="zo")
                        nc.vector.tensor_add(zo, psg[2], xzblk[:, 8:12, tr, :])
                        ro = zpool.tile([128, KT, BC], f32, name="ro", tag="ro")
                        nc.scalar.activation(ro, zo, AF.Relu, bias=half[:, 0:1], scale=0.2)
                        nc.vector.tensor_scalar_min(ro, ro, 1.0)
                        nc.vector.tensor_mul(y_hist[:, t], ro, th)
                        nc.vector.tensor_copy(out=h_bf, in_=y_hist[:, t])

            nc.sync.dma_start(out=y[:], in_=y_hist)
    return nc


def _prep_core_inputs(x, weights, core, Tn=T):
    """weights: dict with all 24 weight arrays (np float32)."""
    d = core // 4
    s = core % 4
    pre = "" if d == 0 else "b"
    gates = ["i", "f", "o", "c"]
    Wc = np.concatenate([weights[f"W{pre}_{g}"] for g in gates], axis=1)
    Uc = np.concatenate([weights[f"U{pre}_{g}"] for g in gates], axis=1)
    bc = np.concatenate([weights[f"b{pre}_{g}"] for g in gates], axis=0)
    xc = x[s * BC : (s + 1) * BC, :Tn]
    if d == 1:
        xc = xc[:, ::-1]
    # [b, t, d] -> [d, t, b] -> [KT, 128, Tn*BC]
    xTc = np.ascontiguousarray(xc.transpose(2, 1, 0)).reshape(KT, 128, Tn * BC)
    return {
        "xT": xTc.astype(ml_dtypes.bfloat16),
        "w": Wc.reshape(KT, 128, 4 * H).astype(ml_dtypes.bfloat16),
        "u": Uc.reshape(KT, 128, 4 * H).astype(ml_dtypes.bfloat16),
        "bias": np.ascontiguousarray(bc.reshape(MT, 128).T).astype(np.float32),
    }


def _gather(results, Tn=T):
    out = np.empty((B, Tn, H), np.float32)
    for s in range(4):
        acc = None
        for d in range(2):
            yc = results[d * 4 + s]["y"]  # [128, Tn, KT, BC]
            part = yc.transpose(3, 1, 2, 0).reshape(BC, Tn, H)
            acc = part if acc is None else acc + part
        out[s * BC : (s + 1) * BC] = acc
    return out


def run(inputs, Tn=T, trace=False):
    import concourse.bacc as bacc
    from concourse.bass_utils import run_bass_kernel_spmd

    x = np.asarray(inputs["x"], np.float32)
    weights = {k: np.asarray(v, np.float32) for k, v in inputs.items() if k != "x"}
    nc = bacc.Bacc("TRN2", target_bir_lowering=False)
    build(nc, Tn)
    nc.compile()
    in_maps = [_prep_core_inputs(x, weights, c, Tn) for c in range(NCORES)]
    res = run_bass_kernel_spmd(nc, in_maps, list(range(NCORES)), trace=trace)
    return _gather(res.results, Tn), res


def kernel(**inputs):
    out, _ = run(inputs)
    return out

